# revision 31
# baseline (speedup 1.0000x reference)
"""CLUB loss kernel for Trainium2 (8 NeuronCores, SPMD row-sharded).

Math: the reference returns mean_i(pos_i - neg_i), a scalar.  Expanding
both terms, the C = sum mu^2*invv piece cancels exactly between pos and
neg, so the kernel only needs six fused reduction passes:

  loss = -0.5/N * (A - 2B) + 0.5/N^2 * (S_invv . S_x2 - 2 S_muinvv . S_x)
      A        = sum_{i,d} x^2 * invv        (scalar)
      B        = sum_{i,d} x * mu * invv     (scalar)
      S_invv   = sum_i invv[i,:]             (d-vector)
      S_muinvv = sum_i mu[i,:]*invv[i,:]     (d-vector)
      S_x      = sum_j x[j,:]                (d-vector)
      S_x2     = sum_j x[j,:]^2              (d-vector)

Each core handles 2048 rows (2 batches of x + matching mu/logvar rows)
and emits f32 partial sums; the host combines them in float64.

Layout: d-major (128, 1024): partition q = (sub-slab b, dim d), free
axis = row index.  Every reduction is a free-axis row-sum riding the
accum_out port of a fused elementwise op.

Schedule (sem-arrival times measured): a 2-slot stream, all 2KB DMA
lines (pow-2 lines matter: 2.5KB lines ran ~2.5us slower, 1KB lines
also regressed):
  sync:   lv full-width int16 (256KB) | x[0:512]  f32   (512KB)
  scalar: mu full-width fp16  (256KB) | x[512:]   f32   (512KB)
Slot-1 (lv+mu, everything the muinvv chain needs) lands ~10.6us,
slot-2 (x) ~12.6us.  ACT: exp halves (so muinvv can start after the
first), then full-width square and Sx copy.  DVE: muinvv halves, then
full-width B and A (the jd junk tile WAW-chains B before A so the
scheduler's optimistic DMA model cannot reorder them).  Quantization:
mu fp16 costs ~3e-3 end-to-end (gate 2e-2); lv int16 Q3.12 costs
~2e-3 because its absolute error is uniform 1.2e-4 (fp16 lv would be
4e-2 - exp amplifies absolute error).
"""

import sys

sys.path.insert(0, "/opt/trn_rl_repo")

import numpy as np
from contextlib import ExitStack

import concourse.bass as bass
import concourse.bacc as bacc
import concourse.tile as tile
from concourse import mybir
from concourse.bass_utils import run_bass_kernel_spmd

F32 = mybir.dt.float32
F16 = mybir.dt.float16
BF16 = mybir.dt.bfloat16
N_CORES = 8
B, D, H, W = 16, 64, 32, 32
HW = H * W                # 1024
N = B * HW                # 16384
NB = B // N_CORES         # 2 sub-slabs (batches) per core
ROWS = NB * HW            # 2048 rows per core
COLS = HW                 # free size of the (128, 1024) layout

SPLIT = 512
SL = [slice(0, SPLIT), slice(SPLIT, COLS)]

# accumulator columns: [Sinvv a,b | Smuinvv a,b | Sx2 | Sx | A | B]
ACC_COLS = 8


def build_nc() -> bass.Bass:
    nc = bacc.Bacc()
    # lv ships as int16 fixed-point (Q3.12): uniform 1.2e-4 abs error ->
    # ~2.4e-3 on the loss (fp16 lv would be 4e-2 — exp amplifies absolute
    # error and fp16 ulp grows with magnitude).  Decode is free: the EXP
    # activation's scale becomes -1/4096.
    lvt = [nc.dram_tensor(f"lvq{p}", [64, COLS], mybir.dt.int16,
                          kind="ExternalInput") for p in range(2)]
    mut = [nc.dram_tensor(f"mup{p}", [64, COLS], F16, kind="ExternalInput")
           for p in range(2)]
    xt = [nc.dram_tensor(f"x{h}", [128, s.stop - s.start], F32,
                         kind="ExternalInput") for h, s in enumerate(SL)]
    accs = nc.dram_tensor("accs", [128, ACC_COLS], F32, kind="ExternalOutput")

    M = mybir.AluOpType.mult
    EXP = mybir.ActivationFunctionType.Exp
    SQ = mybir.ActivationFunctionType.Square
    CP = mybir.ActivationFunctionType.Copy

    with ExitStack() as ctx:
        tc = ctx.enter_context(tile.TileContext(nc))
        big = ctx.enter_context(tc.tile_pool(name="big", bufs=1))
        accp = ctx.enter_context(tc.tile_pool(name="accp", bufs=1))

        xb = big.tile([128, COLS], F32)
        mu = big.tile([128, COLS], F16)
        lv = big.tile([128, COLS], mybir.dt.int16)
        invv = big.tile([128, COLS], F32)
        muinvv = big.tile([128, COLS], F32)
        x2 = big.tile([128, COLS], F32)
        jd = big.tile([128, COLS], BF16)   # shared junk out pins DVE order
        ja = big.tile([128, COLS], BF16)   # ACT junk out for the Sx copy
        acc = accp.tile([128, ACC_COLS], F32)

        # 2-slot stream, all 2KB lines: slot-1 carries lv (int16, full
        # width) on sync and mu (fp16, both partition halves) on scalar;
        # slot-2 carries the f32 x column-halves.  Everything the muinvv
        # chain needs lands at slot-1 (~10.6us), x at slot-2 (~12.1us).
        nc.sync.dma_start(out=lv[0:64, :], in_=lvt[0][:, :])
        nc.scalar.dma_start(out=lv[64:128, :], in_=lvt[1][:, :])
        nc.sync.dma_start(out=mu[0:64, :], in_=mut[0][:, :])
        nc.scalar.dma_start(out=mu[64:128, :], in_=mut[1][:, :])
        nc.sync.dma_start(out=xb[:, SL[0]], in_=xt[0][:, :])
        nc.scalar.dma_start(out=xb[:, SL[1]], in_=xt[1][:, :])

        # ACT: exp halves (int16 in, f32 out, scale folds the Q3.12
        # decode), then full-width square + Sx copy.
        for h in (0, 1):
            s = SL[h]
            nc.scalar.activation(
                out=invv[:, s], in_=lv[:, s], func=EXP, bias=0.0,
                scale=-1.0 / 4096.0,
                accum_out=acc[:, h:h + 1],
            )
        nc.scalar.activation(
            out=x2[:], in_=xb[:], func=SQ, bias=0.0, scale=1.0,
            accum_out=acc[:, 4:5],
        )
        nc.scalar.activation(
            out=ja[:], in_=xb[:], func=CP, bias=0.0, scale=1.0,
            accum_out=acc[:, 5:6],
        )

        # DVE: muinvv halves (start as soon as invv_a + mu land), then
        # full-width B and A (jd WAW keeps B before A).
        for h in (0, 1):
            s = SL[h]
            nc.vector.scalar_tensor_tensor(
                out=muinvv[:, s], in0=mu[:, s], scalar=1.0, in1=invv[:, s],
                op0=M, op1=M, accum_out=acc[:, 2 + h:3 + h],
            )
        nc.vector.scalar_tensor_tensor(
            out=jd[:], in0=xb[:], scalar=1.0, in1=muinvv[:],
            op0=M, op1=M, accum_out=acc[:, 7:8],   # B
        )
        nc.vector.scalar_tensor_tensor(
            out=jd[:], in0=x2[:], scalar=1.0, in1=invv[:],
            op0=M, op1=M, accum_out=acc[:, 6:7],   # A
        )

        nc.sync.dma_start(out=accs[:, :], in_=acc[:])
    return nc


def _ensure_ntff_hook():
    """This image's antenv lacks axon_hooks; if tracing is requested
    (e.g. BASS_TRACE=1), run_bass_kernel_spmd would die on the import.
    Register the ctypes-based hook if available, else a None hook so
    tracing degrades gracefully."""
    import types

    if "antenv.axon_hooks" in sys.modules:
        return
    try:
        import antenv.axon_hooks  # noqa: F401
        return
    except ImportError:
        pass
    hook = None
    try:
        sys.path.insert(0, "/root/.axon_site")
        from trn_agent_boot.trn_boot import _ntff_profile_via_ctypes

        hook = _ntff_profile_via_ctypes("/opt/axon/libaxon_pjrt.so")
    except Exception:
        hook = None
    mod = types.ModuleType("antenv.axon_hooks")
    mod._hook = hook
    mod.get_axon_ntff_profile_hook = lambda: mod._hook
    mod.set_axon_ntff_profile_hook = lambda h: setattr(mod, "_hook", h)
    sys.modules["antenv.axon_hooks"] = mod


_ensure_ntff_hook()

_NC = None


def _get_nc():
    global _NC
    if _NC is None:
        _NC = build_nc()
        # bacc passes legalize multi-sync-wait instructions for TRN2 codegen
        _NC.compile()
    return _NC


def make_in_maps(x, mu, logvar):
    x = np.ascontiguousarray(np.asarray(x, dtype=np.float32))
    mu = np.asarray(mu, dtype=np.float32)
    lv = np.asarray(logvar, dtype=np.float32)
    in_maps = []
    for c in range(N_CORES):
        r0 = c * ROWS
        mu_t = np.concatenate(
            [mu[r0 + b * HW:r0 + (b + 1) * HW].T for b in range(NB)], axis=0
        ).astype(np.float16)
        lv_t = np.concatenate(
            [lv[r0 + b * HW:r0 + (b + 1) * HW].T for b in range(NB)], axis=0
        )
        x_t = x[c * NB:(c + 1) * NB].reshape(128, COLS)
        lv_q = np.clip(np.round(lv_t * 4096.0), -32767, 32767).astype(
            np.int16)
        m = {"mup0": np.ascontiguousarray(mu_t[0:64]),
             "mup1": np.ascontiguousarray(mu_t[64:128]),
             "lvq0": np.ascontiguousarray(lv_q[0:64]),
             "lvq1": np.ascontiguousarray(lv_q[64:128])}
        for h, s in enumerate(SL):
            m[f"x{h}"] = np.ascontiguousarray(x_t[:, s])
        in_maps.append(m)
    return in_maps


def combine(results) -> np.ndarray:
    tot = np.zeros((128, ACC_COLS), dtype=np.float64)
    for r in results:
        tot += np.asarray(r["accs"], dtype=np.float64)
    sinvv = tot[:, 0:2].sum(axis=1).reshape(NB, D).sum(axis=0)
    smuinvv = tot[:, 2:4].sum(axis=1).reshape(NB, D).sum(axis=0)
    sx2 = tot[:, 4].reshape(NB, D).sum(axis=0)
    sx = tot[:, 5].reshape(NB, D).sum(axis=0)
    A = tot[:, 6].sum()
    Bs = tot[:, 7].sum()
    loss = (-0.5 / N * (A - 2.0 * Bs)
            + 0.5 / N**2 * (sinvv @ sx2 - 2.0 * smuinvv @ sx))
    return np.array(loss, dtype=np.float32)


def kernel(x, mu, logvar, **_kwargs):
    nc = _get_nc()
    in_maps = make_in_maps(x, mu, logvar)
    res = run_bass_kernel_spmd(nc, in_maps, list(range(N_CORES)))
    return combine(res.results)


# revision 32
# speedup vs baseline: 1.0308x; 1.0308x over previous
"""CLUB loss kernel for Trainium2 (8 NeuronCores, SPMD row-sharded).

Math: the reference returns mean_i(pos_i - neg_i), a scalar.  Expanding
both terms, the C = sum mu^2*invv piece cancels exactly between pos and
neg, so the kernel only needs six fused reduction passes:

  loss = -0.5/N * (A - 2B) + 0.5/N^2 * (S_invv . S_x2 - 2 S_muinvv . S_x)
      A        = sum_{i,d} x^2 * invv        (scalar)
      B        = sum_{i,d} x * mu * invv     (scalar)
      S_invv   = sum_i invv[i,:]             (d-vector)
      S_muinvv = sum_i mu[i,:]*invv[i,:]     (d-vector)
      S_x      = sum_j x[j,:]                (d-vector)
      S_x2     = sum_j x[j,:]^2              (d-vector)

Each core handles 2048 rows (2 batches of x + matching mu/logvar rows)
and emits f32 partial sums; the host combines them in float64.

Layout: d-major (128, 1024): partition q = (sub-slab b, dim d), free
axis = row index.  Every reduction is a free-axis row-sum riding the
accum_out port of a fused elementwise op.

Schedule (sem-arrival times measured): a 2-slot stream, all 2KB DMA
lines (pow-2 lines matter: 2.5KB lines ran ~2.5us slower, 1KB lines
also regressed):
  sync:   lv full-width int16 (256KB) | x[0:512]  f32   (512KB)
  scalar: mu full-width fp16  (256KB) | x[512:]   f32   (512KB)
Slot-1 (lv+mu, everything the muinvv chain needs) lands ~10.6us,
slot-2 (x) ~12.6us.  ACT: exp halves (so muinvv can start after the
first), then full-width square and Sx copy.  DVE: muinvv halves, then
full-width B and A (the jd junk tile WAW-chains B before A so the
scheduler's optimistic DMA model cannot reorder them).  Quantization:
mu fp16 costs ~3e-3 end-to-end (gate 2e-2); lv int16 Q3.12 costs
~2e-3 because its absolute error is uniform 1.2e-4 (fp16 lv would be
4e-2 - exp amplifies absolute error).
"""

import sys

sys.path.insert(0, "/opt/trn_rl_repo")

import numpy as np
from contextlib import ExitStack

import concourse.bass as bass
import concourse.bacc as bacc
import concourse.tile as tile
from concourse import mybir
from concourse.bass_utils import run_bass_kernel_spmd

F32 = mybir.dt.float32
F16 = mybir.dt.float16
BF16 = mybir.dt.bfloat16
N_CORES = 8
B, D, H, W = 16, 64, 32, 32
HW = H * W                # 1024
N = B * HW                # 16384
NB = B // N_CORES         # 2 sub-slabs (batches) per core
ROWS = NB * HW            # 2048 rows per core
COLS = HW                 # free size of the (128, 1024) layout

SPLIT = 512
SL = [slice(0, SPLIT), slice(SPLIT, COLS)]

# accumulator columns: [Sinvv a,b | Smuinvv a,b | Sx2 | Sx | A | B]
ACC_COLS = 8


def build_nc() -> bass.Bass:
    nc = bacc.Bacc()
    # lv ships as int16 fixed-point (Q3.12): uniform 1.2e-4 abs error ->
    # ~2.4e-3 on the loss (fp16 lv would be 4e-2 — exp amplifies absolute
    # error and fp16 ulp grows with magnitude).  Decode is free: the EXP
    # activation's scale becomes -1/4096.
    lvt = nc.dram_tensor("lvq", [128, COLS], mybir.dt.int16,
                         kind="ExternalInput")
    mut = nc.dram_tensor("muf", [128, COLS], F16, kind="ExternalInput")
    xt = [nc.dram_tensor(f"x{h}", [128, s.stop - s.start], F32,
                         kind="ExternalInput") for h, s in enumerate(SL)]
    accs = nc.dram_tensor("accs", [128, ACC_COLS], F32, kind="ExternalOutput")

    M = mybir.AluOpType.mult
    EXP = mybir.ActivationFunctionType.Exp
    SQ = mybir.ActivationFunctionType.Square
    CP = mybir.ActivationFunctionType.Copy

    with ExitStack() as ctx:
        tc = ctx.enter_context(tile.TileContext(nc))
        big = ctx.enter_context(tc.tile_pool(name="big", bufs=1))
        accp = ctx.enter_context(tc.tile_pool(name="accp", bufs=1))

        xb = big.tile([128, COLS], F32)
        mu = big.tile([128, COLS], F16)
        lv = big.tile([128, COLS], mybir.dt.int16)
        invv = big.tile([128, COLS], F32)
        muinvv = big.tile([128, COLS], F32)
        x2 = big.tile([128, COLS], F32)
        jd = big.tile([128, COLS], BF16)   # shared junk out pins DVE order
        ja = big.tile([128, COLS], BF16)   # ACT junk out for the Sx copy
        acc = accp.tile([128, ACC_COLS], F32)

        # 2-slot stream, all 2KB lines: slot-1 carries lv (int16, full
        # width) on sync and mu (fp16, both partition halves) on scalar;
        # slot-2 carries the f32 x column-halves.  Everything the muinvv
        # chain needs lands at slot-1 (~10.6us), x at slot-2 (~12.1us).
        nc.sync.dma_start(out=lv[:, :], in_=lvt[:, :])
        nc.scalar.dma_start(out=mu[:, :], in_=mut[:, :])
        nc.sync.dma_start(out=xb[:, SL[0]], in_=xt[0][:, :])
        nc.scalar.dma_start(out=xb[:, SL[1]], in_=xt[1][:, :])

        # ACT: exp halves (int16 in, f32 out, scale folds the Q3.12
        # decode), then full-width square + Sx copy.
        for h in (0, 1):
            s = SL[h]
            nc.scalar.activation(
                out=invv[:, s], in_=lv[:, s], func=EXP, bias=0.0,
                scale=-1.0 / 4096.0,
                accum_out=acc[:, h:h + 1],
            )
        nc.scalar.activation(
            out=x2[:], in_=xb[:], func=SQ, bias=0.0, scale=1.0,
            accum_out=acc[:, 4:5],
        )
        nc.scalar.activation(
            out=ja[:], in_=xb[:], func=CP, bias=0.0, scale=1.0,
            accum_out=acc[:, 5:6],
        )

        # DVE: muinvv halves (start as soon as invv_a + mu land), then
        # full-width B and A (jd WAW keeps B before A).
        for h in (0, 1):
            s = SL[h]
            nc.vector.scalar_tensor_tensor(
                out=muinvv[:, s], in0=mu[:, s], scalar=1.0, in1=invv[:, s],
                op0=M, op1=M, accum_out=acc[:, 2 + h:3 + h],
            )
        nc.vector.scalar_tensor_tensor(
            out=jd[:], in0=xb[:], scalar=1.0, in1=muinvv[:],
            op0=M, op1=M, accum_out=acc[:, 7:8],   # B
        )
        nc.vector.scalar_tensor_tensor(
            out=jd[:], in0=x2[:], scalar=1.0, in1=invv[:],
            op0=M, op1=M, accum_out=acc[:, 6:7],   # A
        )

        nc.sync.dma_start(out=accs[:, :], in_=acc[:])
    return nc


def _ensure_ntff_hook():
    """This image's antenv lacks axon_hooks; if tracing is requested
    (e.g. BASS_TRACE=1), run_bass_kernel_spmd would die on the import.
    Register the ctypes-based hook if available, else a None hook so
    tracing degrades gracefully."""
    import types

    if "antenv.axon_hooks" in sys.modules:
        return
    try:
        import antenv.axon_hooks  # noqa: F401
        return
    except ImportError:
        pass
    hook = None
    try:
        sys.path.insert(0, "/root/.axon_site")
        from trn_agent_boot.trn_boot import _ntff_profile_via_ctypes

        hook = _ntff_profile_via_ctypes("/opt/axon/libaxon_pjrt.so")
    except Exception:
        hook = None
    mod = types.ModuleType("antenv.axon_hooks")
    mod._hook = hook
    mod.get_axon_ntff_profile_hook = lambda: mod._hook
    mod.set_axon_ntff_profile_hook = lambda h: setattr(mod, "_hook", h)
    sys.modules["antenv.axon_hooks"] = mod


_ensure_ntff_hook()

_NC = None


def _get_nc():
    global _NC
    if _NC is None:
        _NC = build_nc()
        # bacc passes legalize multi-sync-wait instructions for TRN2 codegen
        _NC.compile()
    return _NC


def make_in_maps(x, mu, logvar):
    x = np.ascontiguousarray(np.asarray(x, dtype=np.float32))
    mu = np.asarray(mu, dtype=np.float32)
    lv = np.asarray(logvar, dtype=np.float32)
    in_maps = []
    for c in range(N_CORES):
        r0 = c * ROWS
        mu_t = np.concatenate(
            [mu[r0 + b * HW:r0 + (b + 1) * HW].T for b in range(NB)], axis=0
        ).astype(np.float16)
        lv_t = np.concatenate(
            [lv[r0 + b * HW:r0 + (b + 1) * HW].T for b in range(NB)], axis=0
        )
        x_t = x[c * NB:(c + 1) * NB].reshape(128, COLS)
        m = {"muf": np.ascontiguousarray(mu_t),
             "lvq": np.ascontiguousarray(np.clip(
                 np.round(lv_t * 4096.0), -32767, 32767).astype(np.int16))}
        for h, s in enumerate(SL):
            m[f"x{h}"] = np.ascontiguousarray(x_t[:, s])
        in_maps.append(m)
    return in_maps


def combine(results) -> np.ndarray:
    tot = np.zeros((128, ACC_COLS), dtype=np.float64)
    for r in results:
        tot += np.asarray(r["accs"], dtype=np.float64)
    sinvv = tot[:, 0:2].sum(axis=1).reshape(NB, D).sum(axis=0)
    smuinvv = tot[:, 2:4].sum(axis=1).reshape(NB, D).sum(axis=0)
    sx2 = tot[:, 4].reshape(NB, D).sum(axis=0)
    sx = tot[:, 5].reshape(NB, D).sum(axis=0)
    A = tot[:, 6].sum()
    Bs = tot[:, 7].sum()
    loss = (-0.5 / N * (A - 2.0 * Bs)
            + 0.5 / N**2 * (sinvv @ sx2 - 2.0 * smuinvv @ sx))
    return np.array(loss, dtype=np.float32)


def kernel(x, mu, logvar, **_kwargs):
    nc = _get_nc()
    in_maps = make_in_maps(x, mu, logvar)
    res = run_bass_kernel_spmd(nc, in_maps, list(range(N_CORES)))
    return combine(res.results)
